# revision 8
# baseline (speedup 1.0000x reference)
"""Sparse (per-query memory) attention kernel for 8 Trainium2 NeuronCores.

Problem shapes (hardcoded):
  x    [2, 8, 128, 512] f32
  mems [2, 8, 128, 64, 512] f32
  mask [2, 8, 128, 64] bool
  Wq [512, 512], Wkv [512, 1024], Wo [512, 512], bo [512]

Sharding: pure data-parallel over the 16 (b, m) slices -> 2 slices/core.

Restructured algorithm (eliminates the kv projection, the FLOP pig):
  sim[i,j,h] = mems[i,j,:] . p[i,h,:]   where p[i,h,:] = Wk_h @ q[i,h]
  attn = softmax_j(sim + mask)          (no max subtraction; logits tiny)
  mbar[i,h,:] = sum_j attn[i,j,h] * mems[i,j,:]
  out = (mbar . Wv_h per head) @ Wo + bo

Device mapping per slice (i=128 queries, j=64 mems, d=512, h=8 heads):
  - scores: fp8 DoubleRow matmuls, stationary p^T [d,(i16,h8)] x moving
    memsT8 [d,(i16,j64)] per 16-query block; diag extracted from PSUM by
    128 tiny [8,64] engine copies (i-col pairing is diagonal).
  - mbar: 8 accumulating matmuls per 8-if group: stationary = zero-padded
    attn window [128=(j64,e2), 128], moving = memsJE [128,(if,d)] bf16;
    contraction packs query-parity e=i%2 to fill 128 partitions.
  - out: mbar -> bf16 -> XBAR transposes -> Wv per head (4-dim moving AP)
    -> Wo, add bias.
"""

import sys

sys.path.insert(0, "/opt/trn_rl_repo")

import numpy as np
import ml_dtypes

B, M, I, J = 2, 8, 128, 64
DIM, HEADS, DIM_HEAD = 512, 8, 64
INNER = HEADS * DIM_HEAD
SCALE = DIM_HEAD**-0.5
NCORES = 8
NSLICE = (B * M) // NCORES  # slices per core = 2
NEG = -1e34  # masked logit, in x128 units (descaled by ACT exp scale)
PS = 128.0  # fp8 p-vector pre-scale (keeps p out of e4m3 subnormals)

TRACE = False
last_results = None

_cache = {}


def _bc(ap, pos, count):
    """Insert a stride-0 (broadcast) dim of `count` at free position `pos`."""
    import concourse.bass as bass

    l = [list(d) for d in ap.ap]
    l.insert(pos, [0, count])
    return bass.AP(tensor=ap.tensor, offset=ap.offset, ap=l)


def _ap(ap, off, dims):
    """Custom AP on the same tensor: free dims [[step,count],...] after the
    partition dim, offset in elements relative to ap's offset."""
    import concourse.bass as bass

    l = [list(ap.ap[0])] + [list(d) for d in dims]
    return bass.AP(tensor=ap.tensor, offset=ap.offset + off, ap=l)


def _build():
    import concourse.tile as tile
    from concourse import bacc, mybir
    import concourse.bass as bass
    from contextlib import ExitStack

    f32 = mybir.dt.float32
    bf16 = mybir.dt.bfloat16
    f8 = mybir.dt.float8e4
    Exp = mybir.ActivationFunctionType.Exp
    DR = mybir.MatmulPerfMode.DoubleRow

    nc = bacc.Bacc("TRN2", target_bir_lowering=False, debug=False, num_devices=NCORES)

    mt8_d = nc.dram_tensor("mt8", [NSLICE * 128, 32768], f8, kind="ExternalInput")
    mje_d = nc.dram_tensor("mje", [NSLICE * 128, 32768], bf16, kind="ExternalInput")
    xt_d = nc.dram_tensor("xt", [NSLICE * DIM, I], bf16, kind="ExternalInput")
    mkb_d = nc.dram_tensor("mkb", [NSLICE * 128, 512], f32, kind="ExternalInput")
    wq_d = nc.dram_tensor("wq", [DIM, INNER], bf16, kind="ExternalInput")
    wkt_d = nc.dram_tensor("wkt", [128, 2048], bf16, kind="ExternalInput")
    wv_d = nc.dram_tensor("wv", [128, 2048], bf16, kind="ExternalInput")
    wo_d = nc.dram_tensor("wo", [INNER, DIM], bf16, kind="ExternalInput")
    bo_d = nc.dram_tensor("bo", [1, DIM], f32, kind="ExternalInput")
    out_d = nc.dram_tensor("out", [NSLICE * I, DIM], f32, kind="ExternalOutput")

    with tile.TileContext(nc) as tc, ExitStack() as ctx:
        const = ctx.enter_context(tc.tile_pool(name="const", bufs=1))
        mt_pool = ctx.enter_context(tc.tile_pool(name="mt", bufs=2))
        mje_pool = ctx.enter_context(tc.tile_pool(name="mje", bufs=3))
        at3_pool = ctx.enter_context(tc.tile_pool(name="at3", bufs=4))
        work = ctx.enter_context(tc.tile_pool(name="work", bufs=2))
        ps_sc = ctx.enter_context(tc.tile_pool(name="pssc", bufs=2, space="PSUM"))
        ps_mb = ctx.enter_context(tc.tile_pool(name="psmb", bufs=2, space="PSUM"))
        ps_misc = ctx.enter_context(tc.tile_pool(name="psmisc", bufs=2, space="PSUM"))

        # --- constant weights ---
        wq_sb = const.tile([128, 4, INNER], bf16)
        nc.sync.dma_start(out=wq_sb, in_=wq_d[:, :].rearrange("(c p) n -> p c n", p=128))
        wkt_sb = const.tile([128, 4, 512], bf16)
        nc.sync.dma_start(out=wkt_sb, in_=wkt_d[:, :].rearrange("p (c n) -> p c n", c=4))
        wv_sb = const.tile([128, 4, 8, 64], bf16)
        nc.sync.dma_start(
            out=wv_sb, in_=wv_d[:, :].rearrange("p (c h n) -> p c h n", c=4, h=8)
        )
        wo_sb = const.tile([128, 4, DIM], bf16)
        nc.sync.dma_start(out=wo_sb, in_=wo_d[:, :].rearrange("(c p) n -> p c n", p=128))
        bo_sb = const.tile([128, DIM], f32)
        nc.sync.dma_start(
            out=bo_sb,
            in_=_ap(bo_d[:, :], 0, [[1, DIM]]).to_broadcast([128, DIM]),
        )

        for s in range(NSLICE):
            # ---------- input DMAs ----------
            mt8 = mt_pool.tile([128, 32768], f8, tag="mt8")
            for c in range(4):
                nc.sync.dma_start(
                    out=mt8[:, c * 8192 : (c + 1) * 8192],
                    in_=mt8_d[s * 128 : (s + 1) * 128, c * 8192 : (c + 1) * 8192],
                )
            xt_sb = work.tile([128, 4, I], bf16, tag="xt")
            nc.sync.dma_start(
                out=xt_sb,
                in_=xt_d[s * DIM : (s + 1) * DIM, :].rearrange("(c p) i -> p c i", p=128),
            )
            mkb_sb = work.tile([128, 512], f32, tag="mkb")
            nc.sync.dma_start(out=mkb_sb, in_=mkb_d[s * 128 : (s + 1) * 128, :])

            # ---------- q = x @ (Wq*scale) ----------
            q_ps = ps_mb.tile([128, INNER], f32, tag="mb")
            for c in range(4):
                nc.tensor.matmul(
                    q_ps, xt_sb[:, c, :], wq_sb[:, c, :], start=(c == 0), stop=(c == 3)
                )
            q_sb = work.tile([128, INNER], bf16, tag="q")
            nc.vector.tensor_copy(out=q_sb, in_=q_ps)
            # qT via XBAR: [i, (h,dh)] -> [(h,dh), i] in 4 chunks
            qt_sb = work.tile([128, 4, I], bf16, tag="qt")
            for c in range(4):
                nc.sync.dma_start(
                    out=qt_sb[:, c, :], in_=q_sb[:, c * 128 : (c + 1) * 128],
                    transpose=True,
                )

            # ---------- p^T = (Wk_h * 128) @ q_h, cast fp8 ----------
            # pt8 free layout: (dc4, iblk8, i16, h8)
            pt8 = work.tile([128, 4096], f8, tag="pt8")
            for h in range(8):
                for dc in range(4):
                    pp = ps_misc.tile([128, 128], f32, tag="pp")
                    hp = h % 2
                    nc.tensor.matmul(
                        pp,
                        wkt_sb[hp * 64 : hp * 64 + 64, h // 2, dc * 128 : (dc + 1) * 128],
                        qt_sb[hp * 64 : hp * 64 + 64, h // 2, :],
                        start=True,
                        stop=True,
                    )
                    eng = nc.vector if (h * 4 + dc) % 2 == 0 else nc.scalar
                    dst = _ap(pt8[:, :], dc * 1024 + h, [[128, 8], [8, 16]])
                    if eng is nc.vector:
                        eng.tensor_copy(out=dst, in_=pp)
                    else:
                        eng.copy(out=dst, in_=pp)

            # ---------- scores (fp8 DoubleRow) + diag extract ----------
            sim_sb = work.tile([128, 8, 64], f32, tag="sim")
            for ib in range(8):
                sc = ps_sc.tile([128, 1024], f32, tag="sc")
                for p in range(2):
                    stat = _ap(
                        pt8[:, :], p * 2048 + ib * 128, [[1024, 2], [1, 128]]
                    )
                    for hf in range(2):
                        mov = _ap(
                            mt8[:, :],
                            p * 16384 + ib * 1024 + hf * 512,
                            [[8192, 2], [1, 512]],
                        )
                        nc.tensor.matmul(
                            sc[:, hf * 512 : (hf + 1) * 512],
                            stat,
                            mov,
                            start=(p == 0),
                            stop=(p == 1),
                            perf_mode=DR,
                            skip_group_check=True,
                        )
                # bulk-evict psum, then diagonal extraction via 16 tiny DMAs
                scE = work.tile([128, 1024], f32, tag="scE")
                if ib % 2 == 0:
                    nc.vector.tensor_copy(out=scE, in_=sc)
                else:
                    nc.scalar.copy(out=scE, in_=sc)
                for i in range(16):
                    nc.sync.dma_start(
                        out=sim_sb[i * 8 : (i + 1) * 8, ib, :],
                        in_=scE[i * 8 : (i + 1) * 8, i * 64 : (i + 1) * 64],
                    )

            # ---------- softmax over j (no max-sub; logits are x128) ----------
            sm = work.tile([128, 512], f32, tag="sm")
            nc.vector.tensor_add(sm, sim_sb[:, :, :].rearrange("p a b -> p (a b)"), mkb_sb)
            att = work.tile([128, 8, 64], bf16, tag="att")
            ssum = work.tile([128, 8], f32, tag="ssum")
            for ib in range(8):
                nc.scalar.activation(
                    out=att[:, ib, :],
                    in_=sm[:, ib * 64 : (ib + 1) * 64],
                    func=Exp,
                    bias=0.0,
                    scale=1.0 / PS,
                    accum_out=ssum[:, ib : ib + 1],
                )
            rsb = work.tile([128, 8], bf16, tag="rsb")
            rs = work.tile([128, 8], f32, tag="rs")
            nc.vector.reciprocal(rs, ssum)
            nc.vector.tensor_copy(out=rsb, in_=rs)
            attn = work.tile([128, 8, 64], bf16, tag="attn")
            nc.vector.tensor_mul(attn, att, _bc(rsb[:, :], 2, 64))

            # ---------- attnT3: transpose + scatter into zeroed windows ----------
            # attn [128=(i16,h8), (ib8, j64)] -> per group g=2a+ib2:
            # at3_g [128=(j64? rows (e2*64=j,e)->  rows = (ib-pair-member, j);
            # cols l*144.. : stationary windows (l*16 offset within l*128 block)
            xb = work.tile([128, 4, 128], bf16, tag="xb")
            for a in range(4):
                nc.sync.dma_start(
                    out=xb[:, a, :],
                    in_=attn[:, 2 * a : 2 * a + 2, :].rearrange("p a b -> p (a b)"),
                    transpose=True,
                )
            at3s = []
            for g in range(8):
                at3 = at3_pool.tile([128, 1024], bf16, tag="at3")
                at3s.append(at3)
                nc.gpsimd.memset(at3, 0)
                a, ib2 = g // 2, g % 2
                for ip in range(2):
                    # src: xb[ib2*64.. , cols i16=2*ifl+ip, h] of chunk a
                    src = _ap(
                        bass_slice_part(xb[:, a, :], ib2 * 64, 64),
                        ip * 8,
                        [[16, 8], [1, 8]],
                    )
                    dst = _ap(
                        bass_slice_part(at3[:, :], ip * 64, 64),
                        ip,
                        [[144, 8], [2, 8]],
                    )
                    nc.vector.tensor_copy(out=dst, in_=src)

            # ---------- mbar: 8 accumulating matmuls per group ----------
            mbE = work.tile([128, 8, 512], bf16, tag="mbE")
            for g in range(8):
                mje_sb = mje_pool.tile([128, 4096], bf16, tag="mje")
                nc.sync.dma_start(
                    out=mje_sb,
                    in_=mje_d[s * 128 : (s + 1) * 128, g * 4096 : (g + 1) * 4096],
                )
                mb = ps_mb.tile([128, 512], f32, tag="mb")
                for l in range(8):
                    nc.tensor.matmul(
                        mb,
                        at3s[g][:, l * 128 : l * 128 + 128],
                        mje_sb[:, l * 512 : (l + 1) * 512],
                        start=(l == 0),
                        stop=(l == 7),
                    )
                eng = nc.vector if g % 2 == 0 else nc.scalar
                if eng is nc.vector:
                    eng.tensor_copy(out=mbE[:, g, :], in_=mb)
                else:
                    eng.copy(out=mbE[:, g, :], in_=mb)

            # ---------- mbar^T via XBAR: 32 blocks ----------
            mbT = work.tile([128, 4, 8, 128], bf16, tag="mbT")
            for g in range(8):
                for dc in range(4):
                    nc.sync.dma_start(
                        out=mbT[:, dc, g, :],
                        in_=mbE[:, g, dc * 128 : (dc + 1) * 128],
                        transpose=True,
                    )

            # ---------- out1 = per-head mbar @ Wv  (psum [dh64*2, i128]) ----------
            o1_sb = work.tile([128, 4, I], bf16, tag="o1")
            for c in range(4):
                o1 = ps_misc.tile([128, 128], f32, tag="pp")
                for hh in range(2):
                    h = c * 2 + hh
                    for dc in range(4):
                        mov = _ap(
                            mbT[:, :, :, :],
                            dc * 1024 + h * 2,
                            [[128, 8], [16, 8], [1, 2]],
                        )
                        nc.tensor.matmul(
                            o1[hh * 64 : (hh + 1) * 64, :],
                            wv_sb[:, dc, h, :],
                            mov,
                            start=(dc == 0),
                            stop=(dc == 3),
                        )
                eng = nc.vector if c % 2 == 0 else nc.scalar
                if eng is nc.vector:
                    eng.tensor_copy(out=o1_sb[:, c, :], in_=o1)
                else:
                    eng.copy(out=o1_sb[:, c, :], in_=o1)

            # ---------- out2 = out1 @ Wo + bo ----------
            fin = ps_mb.tile([128, DIM], f32, tag="mb")
            for c in range(4):
                nc.tensor.matmul(
                    fin, o1_sb[:, c, :], wo_sb[:, c, :], start=(c == 0), stop=(c == 3)
                )
            outb = work.tile([128, DIM], f32, tag="outb")
            nc.vector.tensor_add(outb, fin, bo_sb)
            nc.sync.dma_start(out=out_d[s * I : (s + 1) * I, :], in_=outb)

    nc.compile()
    return nc


def bass_slice_part(ap, p0, cnt):
    """Slice partitions [p0, p0+cnt) of a 2D-ish AP."""
    import concourse.bass as bass

    l = [list(d) for d in ap.ap]
    step = l[0][0]
    l[0] = [step, cnt]
    return bass.AP(tensor=ap.tensor, offset=ap.offset + p0 * step, ap=l)


def kernel(x, mems, mask, Wq, Wkv, Wo, bo):
    from concourse.bass_utils import run_bass_kernel_spmd

    global last_results

    if "nc" not in _cache:
        _cache["nc"] = _build()
    nc = _cache["nc"]

    bf = ml_dtypes.bfloat16
    f8 = ml_dtypes.float8_e4m3
    x = np.asarray(x, dtype=np.float32).reshape(B * M, I, DIM)
    mems = np.asarray(mems, dtype=np.float32).reshape(B * M, I, J, DIM)
    mask = np.asarray(mask).reshape(B * M, I, J)

    # memsT8 fp8: [dsub128, pass2, k2, ib8, i16, j64]
    mt8 = (
        mems.astype(f8)
        .reshape(B * M, 8, 16, J, 2, 2, 128)
        .transpose(0, 6, 4, 5, 1, 2, 3)
        .reshape(B * M, 128, 32768)
    )
    # memsJE bf16: [(e2 j64), (if64 d512)]
    mje = (
        mems.astype(bf)
        .reshape(B * M, 64, 2, J, DIM)
        .transpose(0, 2, 3, 1, 4)
        .reshape(B * M, 128, 32768)
    )
    # xT bf16 [d, i]
    xt = np.ascontiguousarray(x.transpose(0, 2, 1)).astype(bf)
    # mask additive, x128 units, broadcast over h: [(i16,h8), (ib8, j64)]
    mkb = np.where(mask, np.float32(0), np.float32(NEG))  # [S, i, j]
    mkb = mkb.reshape(B * M, 8, 16, J).transpose(0, 2, 1, 3)  # [S, i16, ib, j]
    mkb = np.broadcast_to(mkb[:, :, None, :, :], (B * M, 16, 8, 8, J)).reshape(
        B * M, 128, 512
    )
    mkb = np.ascontiguousarray(mkb)

    Wq_r = (np.asarray(Wq, np.float32) * SCALE).astype(bf)
    Wkv = np.asarray(Wkv, np.float32)
    Wk, Wv = Wkv[:, :INNER], Wkv[:, INNER:]
    # WkT x128: [(hpar2, dh64), c4, D512]: row hp*64+dh, chunk c -> head 2c+hp
    wkt = (
        (Wk * PS)
        .reshape(DIM, 4, 2, 64)  # [D, c, hp, dh]
        .transpose(2, 3, 1, 0)  # [hp, dh, c, D]
        .reshape(128, 2048)
    )
    wkt = np.ascontiguousarray(wkt).astype(bf)
    # Wv: [dsub128, dc4, h8, dh64]
    wv = np.ascontiguousarray(
        Wv.reshape(4, 128, 8, 64).transpose(1, 0, 2, 3)
    ).astype(bf).reshape(128, 2048)
    wo_r = np.asarray(Wo, np.float32).astype(bf)
    bo_r = np.asarray(bo, np.float32).reshape(1, DIM)

    in_maps = []
    for c in range(NCORES):
        sl = slice(NSLICE * c, NSLICE * (c + 1))
        in_maps.append(
            {
                "mt8": mt8[sl].reshape(NSLICE * 128, 32768),
                "mje": mje[sl].reshape(NSLICE * 128, 32768),
                "xt": xt[sl].reshape(NSLICE * DIM, I),
                "mkb": mkb[sl].reshape(NSLICE * 128, 512),
                "wq": Wq_r,
                "wkt": wkt,
                "wv": wv,
                "wo": wo_r,
                "bo": bo_r,
            }
        )

    res = run_bass_kernel_spmd(nc, in_maps, core_ids=list(range(NCORES)), trace=TRACE)
    last_results = res

    out = np.empty((B * M, I, DIM), np.float32)
    for c in range(NCORES):
        o = res.results[c]["out"].reshape(NSLICE, I, DIM)
        out[NSLICE * c : NSLICE * (c + 1)] = o
    return out.reshape(B, M, I, DIM)
